# revision 26
# baseline (speedup 1.0000x reference)
"""MinGRU Trainium2 kernel.

Full-input contract: kernel(**inputs) takes the unsharded inputs
  x [8, 8192, 512] f32, is_init [8] bool (unused by the reference),
  W_hg [1024, 512] f32, W_out [512, 512] f32
and returns (out [8, 8192, 512] f32, h_n [8, 1, 512] f32), matching
reference.py.

Strategy: data-parallel over batch; one NeuronCore per batch element
(8 cores). The log-space Heinsen scan of the reference is algebraically
identical to the linear first-order recurrence
    h_t = c_t * h_{t-1} + v_t,
    c = sigmoid(-gate), v = sigmoid(gate) * g(hidden),
    g(x) = max(x + 0.5, sigmoid(x))        (exact identity)
with all quantities positive and bounded, so it is computed directly in
linear space with the hardware TensorTensorScan instruction (fp32 state).

Per core: x^T arrives pre-transposed [512, 8192] (host-side transpose);
matmul1 produces gate/hidden transposed [h, s] in PSUM (fp32r operands:
one-pass matmuls with ~1.6e-4 accuracy); ACT does the sigmoids out of
PSUM; DVE does g, v and the scan (scan emits fp32r, which is directly
the stationary operand of the output matmul). Output matmul accumulates
over 4 h-blocks into PSUM, ACT copies to SBUF, DMA stores.
"""
import numpy as np
from contextlib import ExitStack

import concourse.bass as bass
import concourse.tile as tile
from concourse import bacc, mybir
from concourse.bass_utils import run_bass_kernel_spmd

dt = mybir.dt
F32 = dt.float32
F32R = dt.float32r
Act = mybir.ActivationFunctionType
Alu = mybir.AluOpType

B, S, D, H = 8, 8192, 512, 512
E = 2 * H            # 1024 (hidden ++ gate)
SB = 1024            # seq superblock
NSB = S // SB        # 8
NHB = H // 128       # 4 h-blocks (partition tiles of H)
NDB = D // 128       # 4 d-blocks (contraction tiles of D)
NSC = SB // 512      # 2 matmul N-chunks per superblock
NST = SB // 128      # 8 output s-tiles per superblock

_CACHE = {}


def build_nc():
    nc = bacc.Bacc("TRN2", target_bir_lowering=False, debug=False)
    xT_d = nc.dram_tensor("xT", [D, S], F32R, kind="ExternalInput")
    whgT_d = nc.dram_tensor("whgT", [D, E], F32R, kind="ExternalInput")
    woutT_d = nc.dram_tensor("woutT", [H, H], F32R, kind="ExternalInput")
    out_d = nc.dram_tensor("out", [S, H], F32, kind="ExternalOutput")
    hn_d = nc.dram_tensor("h_n", [H, 1], F32, kind="ExternalOutput")

    with tile.TileContext(nc) as tc, ExitStack() as ctx:
        wpool = ctx.enter_context(tc.tile_pool(name="w", bufs=1))
        xpool = ctx.enter_context(tc.tile_pool(name="x", bufs=3))
        ew = ctx.enter_context(tc.tile_pool(name="ew", bufs=3))
        hpool = ctx.enter_context(tc.tile_pool(name="h", bufs=16))
        opool = ctx.enter_context(tc.tile_pool(name="o", bufs=4))
        psum_h = ctx.enter_context(tc.tile_pool(name="psh", bufs=2, space="PSUM"))
        psum_g = ctx.enter_context(tc.tile_pool(name="psg", bufs=1, space="PSUM"))
        psum2 = ctx.enter_context(tc.tile_pool(name="ps2", bufs=2, space="PSUM"))

        # PE warm-up: keep TensorE busy during the initial DMA window so the
        # HAM clock-gate reaches full rate before the first real matmul.
        warm = wpool.tile([128, 512], dt.bfloat16, tag="warm")
        nc.vector.memset(warm[:], 1.0)
        for _ in range(16):
            pw = psum2.tile([128, 512], F32, tag="po")
            nc.tensor.matmul(pw[:], warm[:, 0:128], warm[:], start=True, stop=True)

        whgT = []
        for i in range(NDB):
            wt = wpool.tile([128, E], F32R, tag=f"whg{i}")
            whgT.append(wt)
        # First superblock is split so the first real matmul is gated on only
        # 1 MiB of x DMA; the last is split to shorten the serialized tail
        # (mm2 of the final segment waits on the final scan chain).
        segs = []
        s0 = 0
        for seglen in [SB // 2, SB // 2] + [SB] * (NSB - 2) + [SB // 2, SB // 2]:
            segs.append((s0, seglen))
            s0 += seglen

        def emit_mm2(s0, sl, hr_tiles):
            # output matmul for a finished segment
            for st in range(sl // 128):
                po = psum2.tile([128, H], F32, tag="po")
                for j in range(NHB):
                    nc.tensor.matmul(
                        po[:], hr_tiles[j][:, st * 128:(st + 1) * 128],
                        woutT[j][:], start=(j == 0), stop=(j == NHB - 1))
                osb = opool.tile([128, H], F32, tag="osb")
                nc.scalar.copy(osb[:], po[:])
                eng = nc.sync if s0 + sl >= S - SB else nc.gpsimd
                eng.dma_start(
                    out_d[s0 + st * 128:s0 + (st + 1) * 128, :], osb[:])

        woutT = None
        carry = None
        pending = None      # (s0, sl, hr_tiles) of the previous segment
        for s0, sl in segs:
            xT = []
            for i in range(NDB):
                if s0 == 0:
                    # interleave weight-block and x loads: the first matmul
                    # (db=0) is gated on just whgT[0] + xT[0]
                    nc.sync.dma_start(whgT[i][:], whgT_d[128 * i:128 * (i + 1), :])
                t = xpool.tile([128, sl], F32R, tag=f"xt{i}")
                nc.sync.dma_start(t[:], xT_d[128 * i:128 * (i + 1), s0:s0 + sl])
                xT.append(t)
            if woutT is None:
                # deferred: not needed until the first mm2, keeps startup DMA short
                woutT = []
                for i in range(NHB):
                    t = wpool.tile([128, H], F32R, tag=f"wout{i}")
                    nc.sync.dma_start(t[:], woutT_d[128 * i:128 * (i + 1), :])
                    woutT.append(t)

            hr_tiles = []
            next_carry = []
            for j in range(NHB):
                ph = psum_h.tile([128, sl], F32, tag="ph")
                pg = psum_g.tile([128, sl], F32, tag="pg")
                for p, et in ((ph, j), (pg, j + NHB)):
                    for db in range(NDB):
                        for c0 in range(0, sl, 512):
                            cn = min(512, sl - c0)
                            nc.tensor.matmul(
                                p[:, c0:c0 + cn],
                                whgT[db][:, et * 128:(et + 1) * 128],
                                xT[db][:, c0:c0 + cn],
                                start=(db == 0), stop=(db == NDB - 1))
                sig_h = ew.tile([128, sl], F32, tag="sigh")
                nc.scalar.activation(sig_h[:], ph[:], Act.Sigmoid)
                z = ew.tile([128, sl], F32, tag="z")
                nc.scalar.activation(z[:], pg[:], Act.Sigmoid)
                cc = ew.tile([128, sl], F32, tag="c")
                nc.scalar.activation(cc[:], pg[:], Act.Sigmoid, bias=0.0, scale=-1.0)
                g = ew.tile([128, sl], F32, tag="g")
                nc.vector.scalar_tensor_tensor(
                    g[:], ph[:], 0.5, sig_h[:], Alu.add, Alu.max)
                v = ew.tile([128, sl], F32, tag="v")
                nc.vector.tensor_tensor(v[:], z[:], g[:], Alu.mult)
                hr = hpool.tile([128, sl], F32R, tag="hr")
                init = 0.0 if carry is None else carry[j]
                nc.vector.tensor_tensor_scan(
                    hr[:], cc[:], v[:], init, Alu.mult, Alu.add)
                hr_tiles.append(hr)
                next_carry.append(hr[:, sl - 1:sl].bitcast(F32))

            # mm2 runs one segment behind mm1 so the PE's in-order stream
            # never stalls waiting for this segment's scan chain.
            if pending is not None:
                emit_mm2(*pending)
            pending = (s0, sl, hr_tiles)

            if s0 + sl == S:
                emit_mm2(*pending)
                for j in range(NHB):
                    nc.sync.dma_start(
                        hn_d[j * 128:(j + 1) * 128, :], next_carry[j])
            carry = next_carry

    nc.compile()
    return nc


def kernel(**inputs):
    x = np.asarray(inputs["x"], dtype=np.float32)
    W_hg = np.asarray(inputs["W_hg"], dtype=np.float32)
    W_out = np.asarray(inputs["W_out"], dtype=np.float32)
    assert x.shape == (B, S, D), x.shape

    if "nc" not in _CACHE:
        _CACHE["nc"] = build_nc()
    nc = _CACHE["nc"]

    whgT = np.ascontiguousarray(W_hg.T)          # [D, 2H]
    woutT = np.ascontiguousarray(W_out.T)        # [H, O]
    in_maps = [
        {"xT": np.ascontiguousarray(x[b].T), "whgT": whgT, "woutT": woutT}
        for b in range(B)
    ]
    r = run_bass_kernel_spmd(nc, in_maps, list(range(B)))
    out = np.stack([np.asarray(r.results[b]["out"]) for b in range(B)])
    h_n = np.stack([np.asarray(r.results[b]["h_n"]).reshape(1, H) for b in range(B)])
    return out, h_n


# revision 27
# speedup vs baseline: 1.0369x; 1.0369x over previous
"""MinGRU Trainium2 kernel.

Full-input contract: kernel(**inputs) takes the unsharded inputs
  x [8, 8192, 512] f32, is_init [8] bool (unused by the reference),
  W_hg [1024, 512] f32, W_out [512, 512] f32
and returns (out [8, 8192, 512] f32, h_n [8, 1, 512] f32), matching
reference.py.

Strategy: data-parallel over batch; one NeuronCore per batch element
(8 cores). The log-space Heinsen scan of the reference is algebraically
identical to the linear first-order recurrence
    h_t = c_t * h_{t-1} + v_t,
    c = sigmoid(-gate), v = sigmoid(gate) * g(hidden),
    g(x) = max(x + 0.5, sigmoid(x))        (exact identity)
with all quantities positive and bounded, so it is computed directly in
linear space with the hardware TensorTensorScan instruction (fp32 state).

Per core: x^T arrives pre-transposed [512, 8192] (host-side transpose);
matmul1 produces gate/hidden transposed [h, s] in PSUM (fp32r operands:
one-pass matmuls with ~1.6e-4 accuracy); ACT does the sigmoids out of
PSUM; DVE does g, v and the scan (scan emits fp32r, which is directly
the stationary operand of the output matmul). Output matmul accumulates
over 4 h-blocks into PSUM, ACT copies to SBUF, DMA stores.
"""
import numpy as np
from contextlib import ExitStack

import concourse.bass as bass
import concourse.tile as tile
from concourse import bacc, mybir
from concourse.bass_utils import run_bass_kernel_spmd

dt = mybir.dt
F32 = dt.float32
F32R = dt.float32r
Act = mybir.ActivationFunctionType
Alu = mybir.AluOpType

B, S, D, H = 8, 8192, 512, 512
E = 2 * H            # 1024 (hidden ++ gate)
SB = 1024            # seq superblock
NSB = S // SB        # 8
NHB = H // 128       # 4 h-blocks (partition tiles of H)
NDB = D // 128       # 4 d-blocks (contraction tiles of D)
NSC = SB // 512      # 2 matmul N-chunks per superblock
NST = SB // 128      # 8 output s-tiles per superblock

_CACHE = {}


def build_nc():
    nc = bacc.Bacc("TRN2", target_bir_lowering=False, debug=False)
    xT_d = nc.dram_tensor("xT", [D, S], F32R, kind="ExternalInput")
    whgT_d = nc.dram_tensor("whgT", [D, E], F32R, kind="ExternalInput")
    woutT_d = nc.dram_tensor("woutT", [H, H], F32R, kind="ExternalInput")
    out_d = nc.dram_tensor("out", [S, H], F32, kind="ExternalOutput")
    hn_d = nc.dram_tensor("h_n", [H, 1], F32, kind="ExternalOutput")

    with tile.TileContext(nc) as tc, ExitStack() as ctx:
        wpool = ctx.enter_context(tc.tile_pool(name="w", bufs=1))
        xpool = ctx.enter_context(tc.tile_pool(name="x", bufs=3))
        ew = ctx.enter_context(tc.tile_pool(name="ew", bufs=3))
        hpool = ctx.enter_context(tc.tile_pool(name="h", bufs=16))
        opool = ctx.enter_context(tc.tile_pool(name="o", bufs=4))
        psum_h = ctx.enter_context(tc.tile_pool(name="psh", bufs=2, space="PSUM"))
        psum_g = ctx.enter_context(tc.tile_pool(name="psg", bufs=1, space="PSUM"))
        psum2 = ctx.enter_context(tc.tile_pool(name="ps2", bufs=2, space="PSUM"))

        # PE warm-up: keep TensorE busy during the initial DMA window so the
        # HAM clock-gate reaches full rate before the first real matmul.
        warm = wpool.tile([128, 512], dt.bfloat16, tag="warm")
        nc.vector.memset(warm[:], 1.0)
        for _ in range(16):
            pw = psum2.tile([128, 512], F32, tag="po")
            nc.tensor.matmul(pw[:], warm[:, 0:128], warm[:], start=True, stop=True)

        whgT = []
        for i in range(NDB):
            wt = wpool.tile([128, E], F32R, tag=f"whg{i}")
            whgT.append(wt)
        # First superblock is split so the first real matmul is gated on only
        # 1 MiB of x DMA; the last is split to shorten the serialized tail
        # (mm2 of the final segment waits on the final scan chain).
        segs = []
        s0 = 0
        for seglen in [SB // 2, SB // 2] + [SB] * (NSB - 2) + [SB // 2, SB // 2]:
            segs.append((s0, seglen))
            s0 += seglen

        def emit_mm2(s0, sl, hr_tiles, interleave=False):
            # output matmul for a finished segment
            nst = sl // 128
            sts = [(st,) for st in range(nst)]
            if interleave:
                # final segment: pair st-groups j-outer so early matmuls
                # overlap the still-running scan chain instead of all 16
                # waiting on the last h-block's scan
                sts = [tuple(range(i, min(i + 2, nst))) for i in range(0, nst, 2)]
            for group in sts:
                pos = []
                for st in group:
                    po = psum2.tile([128, H], F32, tag="po", name=f"po_{s0}_{st}")
                    pos.append(po)
                for j in range(NHB):
                    for st, po in zip(group, pos):
                        nc.tensor.matmul(
                            po[:], hr_tiles[j][:, st * 128:(st + 1) * 128],
                            woutT[j][:], start=(j == 0), stop=(j == NHB - 1))
                for st, po in zip(group, pos):
                    osb = opool.tile([128, H], F32, tag="osb")
                    nc.scalar.copy(osb[:], po[:])
                    eng = nc.sync if s0 + sl >= S - SB else nc.gpsimd
                    eng.dma_start(
                        out_d[s0 + st * 128:s0 + (st + 1) * 128, :], osb[:])

        woutT = None
        carry = None
        pending = None      # (s0, sl, hr_tiles) of the previous segment
        for s0, sl in segs:
            xT = []
            for i in range(NDB):
                if s0 == 0:
                    # interleave weight-block and x loads: the first matmul
                    # (db=0) is gated on just whgT[0] + xT[0]
                    nc.sync.dma_start(whgT[i][:], whgT_d[128 * i:128 * (i + 1), :])
                t = xpool.tile([128, sl], F32R, tag=f"xt{i}")
                nc.sync.dma_start(t[:], xT_d[128 * i:128 * (i + 1), s0:s0 + sl])
                xT.append(t)
            if woutT is None:
                # deferred: not needed until the first mm2, keeps startup DMA short
                woutT = []
                for i in range(NHB):
                    t = wpool.tile([128, H], F32R, tag=f"wout{i}")
                    nc.sync.dma_start(t[:], woutT_d[128 * i:128 * (i + 1), :])
                    woutT.append(t)

            hr_tiles = []
            next_carry = []
            for j in range(NHB):
                ph = psum_h.tile([128, sl], F32, tag="ph")
                pg = psum_g.tile([128, sl], F32, tag="pg")
                for p, et in ((ph, j), (pg, j + NHB)):
                    for db in range(NDB):
                        for c0 in range(0, sl, 512):
                            cn = min(512, sl - c0)
                            nc.tensor.matmul(
                                p[:, c0:c0 + cn],
                                whgT[db][:, et * 128:(et + 1) * 128],
                                xT[db][:, c0:c0 + cn],
                                start=(db == 0), stop=(db == NDB - 1))
                sig_h = ew.tile([128, sl], F32, tag="sigh")
                nc.scalar.activation(sig_h[:], ph[:], Act.Sigmoid)
                z = ew.tile([128, sl], F32, tag="z")
                nc.scalar.activation(z[:], pg[:], Act.Sigmoid)
                cc = ew.tile([128, sl], F32, tag="c")
                nc.scalar.activation(cc[:], pg[:], Act.Sigmoid, bias=0.0, scale=-1.0)
                g = ew.tile([128, sl], F32, tag="g")
                nc.vector.scalar_tensor_tensor(
                    g[:], ph[:], 0.5, sig_h[:], Alu.add, Alu.max)
                v = ew.tile([128, sl], F32, tag="v")
                nc.vector.tensor_tensor(v[:], z[:], g[:], Alu.mult)
                hr = hpool.tile([128, sl], F32R, tag="hr")
                init = 0.0 if carry is None else carry[j]
                nc.vector.tensor_tensor_scan(
                    hr[:], cc[:], v[:], init, Alu.mult, Alu.add)
                hr_tiles.append(hr)
                next_carry.append(hr[:, sl - 1:sl].bitcast(F32))

            # mm2 runs one segment behind mm1 so the PE's in-order stream
            # never stalls waiting for this segment's scan chain.
            if pending is not None:
                emit_mm2(*pending)
            pending = (s0, sl, hr_tiles)

            if s0 + sl == S:
                emit_mm2(*pending, interleave=True)
                for j in range(NHB):
                    nc.sync.dma_start(
                        hn_d[j * 128:(j + 1) * 128, :], next_carry[j])
            carry = next_carry

    nc.compile()
    return nc


def kernel(**inputs):
    x = np.asarray(inputs["x"], dtype=np.float32)
    W_hg = np.asarray(inputs["W_hg"], dtype=np.float32)
    W_out = np.asarray(inputs["W_out"], dtype=np.float32)
    assert x.shape == (B, S, D), x.shape

    if "nc" not in _CACHE:
        _CACHE["nc"] = build_nc()
    nc = _CACHE["nc"]

    whgT = np.ascontiguousarray(W_hg.T)          # [D, 2H]
    woutT = np.ascontiguousarray(W_out.T)        # [H, O]
    in_maps = [
        {"xT": np.ascontiguousarray(x[b].T), "whgT": whgT, "woutT": woutT}
        for b in range(B)
    ]
    r = run_bass_kernel_spmd(nc, in_maps, list(range(B)))
    out = np.stack([np.asarray(r.results[b]["out"]) for b in range(B)])
    h_n = np.stack([np.asarray(r.results[b]["h_n"]).reshape(1, H) for b in range(B)])
    return out, h_n
